# revision 9
# baseline (speedup 1.0000x reference)
"""MoE layer (noisy top-2 routing, 8 experts) on 8 TRN2 NeuronCores.

Two device strategies (host picks via MOE_MODE env, default "sparse"):

* sparse (expert parallelism with host-side token dispatch): the host runs the
  router in fp64 purely to decide token->expert placement (the math itself is
  recomputed on device), gathers each expert's ~T/4 tokens, and each core runs
  its expert's FFN on the gathered tokens plus the router for its 512-token
  slice (which produces the returned gate). The host scatter-adds the two
  gate-weighted expert outputs per token. Exploits the top-2 sparsity: 4x
  fewer FLOPs than the dense formulation.

* dense (data parallelism): each core takes T/8 = 512 tokens and runs the
  router + all 8 experts' FFNs on its slice, combining with the on-device
  gate. No host-side math at all; exact replica of the reference dense
  formulation.

Router matmuls run in plain fp32 (top-2 selection needs it: the tightest
2nd/3rd margin is ~2e-5, fp32r noise is ~1e-4). FFN matmuls run in fp32r
(full PE rate, ~1e-4 relative error).

Self-contained: hardcodes shapes from the problem spec.
"""

import os
import sys

import numpy as np

for _p in ("/opt/trn_rl_repo", "/opt/pypackages"):
    if _p not in sys.path:
        sys.path.append(_p)

import concourse.mybir as mybir  # noqa: E402
import concourse.tile as tile  # noqa: E402
from concourse import bacc  # noqa: E402
from concourse.bass_utils import run_bass_kernel_spmd  # noqa: E402

D = 1024  # d_model
H = 4096  # d_ff
E = 8  # experts
NCORES = 8
T = 4096  # total tokens
TL = T // NCORES  # router tokens per core
NT = TL // 128  # router token tiles per core
KD = D // 128  # contraction tiles over d_model
NH = H // 128  # H tiles
F32 = mybir.dt.float32
F32R = mybir.dt.float32r
AX = mybir.AxisListType
ALU = mybir.AluOpType
ACT = mybir.ActivationFunctionType

_cache: dict = {}


def _emit_router(nc, apool, pspool, xT_sb, rw_sb, rb_sb, nz_sb, gate_sb, gout_v):
    """Noisy top-2 router for NT tiles of 128 tokens; writes gate to gout and
    gate_sb. Plain fp32 matmuls."""
    for t in range(NT):
        ps_r = pspool.tile(
            [128, 2 * E], F32, name="ps", tag="ps", padded_shape=[128, 512]
        )
        for k in range(KD):
            nc.tensor.matmul(
                ps_r[:],
                xT_sb[:, k, t * 128 : (t + 1) * 128],
                rw_sb[:, k, :],
                start=(k == 0),
                stop=(k == KD - 1),
            )
        lg = apool.tile([128, 2 * E], F32, name="lg", tag="lg")
        nc.vector.tensor_add(lg[:], ps_r[:], rb_sb[:])

        # softplus(z) = relu(z) + ln(1 + exp(-|z|)) via exp/ln tables
        az = apool.tile([128, E], F32, name="az", tag="az")
        nc.scalar.activation(az[:], lg[:, E:], ACT.Abs)
        en = apool.tile([128, E], F32, name="en", tag="en")
        nc.scalar.activation(en[:], az[:], ACT.Exp, scale=-1.0)
        nc.vector.tensor_scalar(en[:], en[:], 1.0, None, ALU.add)
        lnv = apool.tile([128, E], F32, name="lnv", tag="lnv")
        nc.scalar.activation(lnv[:], en[:], ACT.Ln)
        rz = apool.tile([128, E], F32, name="rz", tag="rz")
        nc.scalar.activation(rz[:], lg[:, E:], ACT.Relu)
        ns = apool.tile([128, E], F32, name="ns", tag="ns")
        nc.vector.tensor_add(ns[:], lnv[:], rz[:])
        noisy = apool.tile([128, E], F32, name="noisy", tag="noisy")
        nc.vector.tensor_mul(noisy[:], nz_sb[:, t, :], ns[:])
        nc.vector.tensor_add(noisy[:], noisy[:], lg[:, :E])

        m1 = apool.tile([128, 1], F32, name="m1", tag="m1")
        nc.vector.reduce_max(m1[:], noisy[:], axis=AX.X)
        eq = apool.tile([128, E], F32, name="eq", tag="eq")
        nc.vector.tensor_scalar(eq[:], noisy[:], m1[:], None, ALU.is_equal)
        v2 = apool.tile([128, E], F32, name="v2", tag="v2")
        nc.vector.scalar_tensor_tensor(
            v2[:], eq[:], -1e30, noisy[:], op0=ALU.mult, op1=ALU.add
        )
        m2 = apool.tile([128, 1], F32, name="m2", tag="m2")
        nc.vector.reduce_max(m2[:], v2[:], axis=AX.X)

        negm1 = apool.tile([128, 1], F32, name="negm1", tag="negm1")
        nc.vector.tensor_scalar(negm1[:], m1[:], -1.0, None, ALU.mult)
        expv = apool.tile([128, E], F32, name="expv", tag="expv")
        nc.scalar.activation(expv[:], noisy[:], ACT.Exp, bias=negm1[:])
        # r = sigmoid(m1 - m2) = 1 / (1 + exp(m2 - m1))
        d21 = apool.tile([128, 1], F32, name="d21", tag="d21")
        nc.vector.tensor_sub(d21[:], m2[:], m1[:])
        er = apool.tile([128, 1], F32, name="er", tag="er")
        nc.scalar.activation(er[:], d21[:], ACT.Exp)
        nc.vector.tensor_scalar(er[:], er[:], 1.0, None, ALU.add)
        r = apool.tile([128, 1], F32, name="r", tag="r")
        nc.vector.reciprocal(r[:], er[:])
        ge = apool.tile([128, E], F32, name="ge", tag="ge")
        nc.vector.tensor_scalar(ge[:], noisy[:], m2[:], None, ALU.is_ge)
        nc.vector.tensor_mul(expv[:], expv[:], ge[:])
        nc.vector.tensor_scalar(gate_sb[:, t, :], expv[:], r[:], None, ALU.mult)

        nc.sync.dma_start(gout_v[t], gate_sb[:, t, :])


def _chunks(C):
    out, off = [], 0
    while off < C:
        w = min(512, C - off)
        out.append((off, w))
        off += w
    return out


def _build_sparse_nc(C):
    """SPMD program: router on a 512-token slice + one expert's FFN on C
    pre-gathered tokens."""
    nc = bacc.Bacc(None, target_bir_lowering=False)

    xTr = nc.dram_tensor("xTr", [D, TL], F32, kind="ExternalInput")
    nz = nc.dram_tensor("nz", [TL, E], F32, kind="ExternalInput")
    rw = nc.dram_tensor("rw", [D, 2 * E], F32, kind="ExternalInput")
    rb = nc.dram_tensor("rb", [128, 2 * E], F32, kind="ExternalInput")
    xTg = nc.dram_tensor("xTg", [D, C], F32R, kind="ExternalInput")
    gv = nc.dram_tensor("gv", [1, C], F32R, kind="ExternalInput")
    w1 = nc.dram_tensor("w1", [D, H], F32R, kind="ExternalInput")
    b1c = nc.dram_tensor("b1c", [128, NH], F32, kind="ExternalInput")
    w2 = nc.dram_tensor("w2", [H, D], F32R, kind="ExternalInput")
    b2 = nc.dram_tensor("b2", [1, D], F32R, kind="ExternalInput")
    yout = nc.dram_tensor("yout", [C, D], F32, kind="ExternalOutput")
    gout = nc.dram_tensor("gout", [TL, E], F32, kind="ExternalOutput")

    yout_v = yout.rearrange("(t p) d -> t p d", p=128)
    gout_v = gout.rearrange("(t p) e -> t p e", p=128)
    CT = C // 128  # gathered token tiles

    with tile.TileContext(nc) as tc:
        with (
            tc.tile_pool(name="const", bufs=1) as cpool,
            tc.tile_pool(name="acts", bufs=1) as apool,
            tc.tile_pool(name="hbuf", bufs=36) as hpool,
            tc.tile_pool(name="w1buf", bufs=4) as w1pool,
            tc.tile_pool(name="w2buf", bufs=4) as w2pool,
            tc.tile_pool(name="ybuf", bufs=4) as ypool,
            tc.tile_pool(name="ps", bufs=8, space="PSUM") as pspool,
        ):
            xT_sb = apool.tile([128, KD, TL], F32)
            nc.sync.dma_start(xT_sb[:], xTr.rearrange("(k p) t -> p k t", p=128))
            rw_sb = cpool.tile([128, KD, 2 * E], F32)
            nc.sync.dma_start(rw_sb[:], rw.rearrange("(k p) c -> p k c", p=128))
            rb_sb = cpool.tile([128, 2 * E], F32)
            nc.sync.dma_start(rb_sb[:], rb[:])
            nz_sb = apool.tile([128, NT, E], F32)
            nc.sync.dma_start(nz_sb[:], nz.rearrange("(t p) e -> p t e", p=128))
            gate_sb = apool.tile([128, NT, E], F32)

            xTg_sb = apool.tile([128, KD, C], F32R)
            nc.sync.dma_start(xTg_sb[:], xTg.rearrange("(k p) t -> p k t", p=128))
            gvc_sb = apool.tile([128, CT], F32)
            nc.sync.dma_start(
                gvc_sb[:], gv.bitcast(F32).rearrange("o (t p) -> (o p) t", p=128)
            )
            gvr_sb = cpool.tile([1, C], F32R)
            nc.sync.dma_start(gvr_sb[:], gv[:])
            b1_sb = cpool.tile([128, NH], F32)
            nc.sync.dma_start(b1_sb[:], b1c[:])
            b2_sb = cpool.tile([1, D], F32R)
            nc.sync.dma_start(b2_sb[:], b2[:])

            _emit_router(nc, apool, pspool, xT_sb, rw_sb, rb_sb, nz_sb, gate_sb, gout_v)

            for coff, cw in _chunks(C):
                ct0 = coff // 128
                # ---- h^T = relu(w1.T @ xTg + b1) for this token chunk ----
                h_tiles = []
                for g in range(NH // 4):
                    psh = [
                        pspool.tile([128, 512], F32, name="ps", tag="ps")
                        for _ in range(4)
                    ]
                    for k in range(KD):
                        w1t = w1pool.tile([128, 512], F32R, name="w1t", tag="w1t")
                        nc.sync.dma_start(
                            w1t[:],
                            w1[k * 128 : (k + 1) * 128, g * 512 : (g + 1) * 512],
                        )
                        for i4 in range(4):
                            nc.tensor.matmul(
                                psh[i4][:, :cw],
                                w1t[:, i4 * 128 : (i4 + 1) * 128],
                                xTg_sb[:, k, coff : coff + cw],
                                start=(k == 0),
                                stop=(k == KD - 1),
                            )
                    for i4 in range(4):
                        i = g * 4 + i4
                        ht = hpool.tile([128, 512], F32R, name="ht", tag="ht")
                        nc.scalar.activation(
                            ht[:, :cw], psh[i4][:, :cw], ACT.Relu,
                            bias=b1_sb[:, i : i + 1],
                        )
                        h_tiles.append(ht)

                # ---- y = gv * (h @ w2 + b2), token-major ----
                ntt = cw // 128
                for d in range(2):
                    psy = [
                        pspool.tile([128, 512], F32, name="ps", tag="ps")
                        for _ in range(ntt)
                    ]
                    for i in range(NH):
                        w2t = w2pool.tile([128, 512], F32R, name="w2t", tag="w2t")
                        nc.sync.dma_start(
                            w2t[:],
                            w2[i * 128 : (i + 1) * 128, d * 512 : (d + 1) * 512],
                        )
                        for tt in range(ntt):
                            nc.tensor.matmul(
                                psy[tt][:],
                                h_tiles[i][:, tt * 128 : (tt + 1) * 128],
                                w2t[:],
                                start=(i == 0),
                                stop=False,
                            )
                    for tt in range(ntt):
                        # gated b2 term: psy += gv[:,tile].T @ b2 (rank-1, fp32)
                        nc.tensor.matmul(
                            psy[tt][:],
                            gvr_sb[:, coff + tt * 128 : coff + (tt + 1) * 128],
                            b2_sb[:, d * 512 : (d + 1) * 512],
                            start=False,
                            stop=True,
                        )
                        y_sb = ypool.tile([128, 512], F32, name="y_sb", tag="y_sb")
                        nc.vector.tensor_scalar(
                            y_sb[:], psy[tt][:], gvc_sb[:, ct0 + tt : ct0 + tt + 1],
                            None, ALU.mult,
                        )
                        nc.sync.dma_start(
                            yout_v[ct0 + tt][:, d * 512 : (d + 1) * 512], y_sb[:]
                        )

    nc.compile()
    return nc


def _build_dense_nc():
    """SPMD program: router + dense 8-expert FFN for TL tokens."""
    nc = bacc.Bacc(None, target_bir_lowering=False)

    xT = nc.dram_tensor("xT", [D, TL], F32R, kind="ExternalInput")
    nz = nc.dram_tensor("nz", [TL, E], F32, kind="ExternalInput")
    rw = nc.dram_tensor("rw", [D, 2 * E], F32, kind="ExternalInput")
    rb = nc.dram_tensor("rb", [128, 2 * E], F32, kind="ExternalInput")
    w1s = nc.dram_tensor("w1s", [E, D, H], F32R, kind="ExternalInput")
    b1r = nc.dram_tensor("b1r", [128, E, NH], F32, kind="ExternalInput")
    w2s = nc.dram_tensor("w2s", [E, H, D], F32R, kind="ExternalInput")
    b2s = nc.dram_tensor("b2s", [E, D], F32, kind="ExternalInput")
    outp = nc.dram_tensor("outp", [TL, D], F32, kind="ExternalOutput")
    gout = nc.dram_tensor("gout", [TL, E], F32, kind="ExternalOutput")

    out_v = outp.rearrange("(t p) d -> t p d", p=128)
    gout_v = gout.rearrange("(t p) e -> t p e", p=128)

    with tile.TileContext(nc) as tc:
        with (
            tc.tile_pool(name="const", bufs=1) as cpool,
            tc.tile_pool(name="acts", bufs=1) as apool,
            tc.tile_pool(name="hbuf", bufs=36) as hpool,
            tc.tile_pool(name="w1buf", bufs=3) as w1pool,
            tc.tile_pool(name="w2buf", bufs=3) as w2pool,
            tc.tile_pool(name="accbuf", bufs=8) as accpool,
            tc.tile_pool(name="ps", bufs=8, space="PSUM") as pspool,
        ):
            from concourse.masks import make_identity

            xT_sb = apool.tile([128, KD, TL], F32)
            nc.sync.dma_start(
                xT_sb[:], xT.bitcast(F32).rearrange("(k p) t -> p k t", p=128)
            )
            xTr_sb = apool.tile([128, KD, TL], F32R)
            nc.sync.dma_start(xTr_sb[:], xT.rearrange("(k p) t -> p k t", p=128))
            rw_sb = cpool.tile([128, KD, 2 * E], F32)
            nc.sync.dma_start(rw_sb[:], rw.rearrange("(k p) c -> p k c", p=128))
            rb_sb = cpool.tile([128, 2 * E], F32)
            nc.sync.dma_start(rb_sb[:], rb[:])
            nz_sb = apool.tile([128, NT, E], F32)
            nc.sync.dma_start(nz_sb[:], nz.rearrange("(t p) e -> p t e", p=128))
            b1_sb = cpool.tile([128, E, NH], F32)
            nc.sync.dma_start(b1_sb[:], b1r[:])
            b2_sb = cpool.tile([E, D], F32)
            nc.sync.dma_start(b2_sb[:], b2s[:])
            ident = cpool.tile([128, 128], F32)
            make_identity(nc, ident[:])

            gate_sb = apool.tile([128, NT, E], F32)
            gT_sb = apool.tile([E, TL], F32)

            _emit_router(nc, apool, pspool, xT_sb, rw_sb, rb_sb, nz_sb, gate_sb, gout_v)

            # gate transposed (for the gate @ b2 acc init)
            for t in range(NT):
                ps_gt = pspool.tile(
                    [E, 128], F32, name="ps", tag="ps", padded_shape=[128, 512]
                )
                nc.tensor.transpose(ps_gt[:], gate_sb[:, t, :], ident[:])
                nc.vector.tensor_copy(gT_sb[:, t * 128 : (t + 1) * 128], ps_gt[:])

            # ---- acc init with the gate-weighted b2 term: acc = gate @ b2s ----
            acc = {}
            for t in range(NT):
                for d in range(2):
                    ps = pspool.tile([128, 512], F32, name="ps", tag="ps")
                    nc.tensor.matmul(
                        ps[:],
                        gT_sb[:, t * 128 : (t + 1) * 128],
                        b2_sb[:, d * 512 : (d + 1) * 512],
                        start=True,
                        stop=True,
                    )
                    a = accpool.tile([128, 512], F32, name="a", tag="a")
                    nc.vector.tensor_copy(a[:], ps[:])
                    acc[(t, d)] = a

            # ---- experts ----
            for e in range(E):
                h_tiles = []
                for g in range(NH // 4):  # groups of 4 H-tiles (512 cols of w1)
                    psh = [
                        pspool.tile([128, 512], F32, name="ps", tag="ps")
                        for _ in range(4)
                    ]
                    for k in range(KD):
                        w1t = w1pool.tile([128, 512], F32R, name="w1t", tag="w1t")
                        nc.sync.dma_start(
                            w1t[:],
                            w1s[e, k * 128 : (k + 1) * 128, g * 512 : (g + 1) * 512],
                        )
                        for i4 in range(4):
                            nc.tensor.matmul(
                                psh[i4][:],
                                w1t[:, i4 * 128 : (i4 + 1) * 128],
                                xTr_sb[:, k, :],
                                start=(k == 0),
                                stop=(k == KD - 1),
                            )
                    for i4 in range(4):
                        i = g * 4 + i4
                        ht = hpool.tile([128, TL], F32R, name="ht", tag="ht")
                        nc.scalar.activation(
                            ht[:], psh[i4][:], ACT.Relu, bias=b1_sb[:, e, i : i + 1]
                        )
                        h_tiles.append(ht)

                for d in range(2):
                    psy = [
                        pspool.tile([128, 512], F32, name="ps", tag="ps")
                        for _ in range(NT)
                    ]
                    for i in range(NH):
                        w2t = w2pool.tile([128, 512], F32R, name="w2t", tag="w2t")
                        nc.sync.dma_start(
                            w2t[:],
                            w2s[e, i * 128 : (i + 1) * 128, d * 512 : (d + 1) * 512],
                        )
                        for t in range(NT):
                            nc.tensor.matmul(
                                psy[t][:],
                                h_tiles[i][:, t * 128 : (t + 1) * 128],
                                w2t[:],
                                start=(i == 0),
                                stop=(i == NH - 1),
                            )
                    for t in range(NT):
                        nc.vector.scalar_tensor_tensor(
                            acc[(t, d)][:],
                            psy[t][:],
                            gate_sb[:, t, e : e + 1],
                            acc[(t, d)][:],
                            op0=ALU.mult,
                            op1=ALU.add,
                        )

            for t in range(NT):
                for d in range(2):
                    nc.sync.dma_start(
                        out_v[t][:, d * 512 : (d + 1) * 512], acc[(t, d)][:]
                    )

    nc.compile()
    return nc


def _get_nc():
    if "dense" not in _cache:
        _cache["dense"] = _build_dense_nc()
    return _cache["dense"]


def _get_sparse_nc(C):
    key = ("sparse", C)
    if key not in _cache:
        _cache[key] = _build_sparse_nc(C)
    return _cache[key]


def _marshal(inputs):
    f = lambda k: np.ascontiguousarray(np.asarray(inputs[k], dtype=np.float32))
    hs = f("hidden_states").reshape(T, D)
    noise = f("noise")
    rw_cat = np.concatenate([f("route_w"), f("noise_w")], axis=1)  # [D, 16]
    rb_cat = np.broadcast_to(
        np.concatenate([f("route_b"), f("noise_b")])[None, :], (128, 2 * E)
    ).copy()
    w1 = f("w1")
    b1r = f("b1").reshape(E, NH, 128).transpose(2, 0, 1).copy()  # [128, E, NH]
    w2 = f("w2")
    b2 = f("b2")
    xT = np.ascontiguousarray(hs.T)  # [D, T]
    return hs, noise, rw_cat, rb_cat, w1, b1r, w2, b2, xT


def _host_route(hs, noise, inputs):
    """fp64 replica of the router, used only to place tokens on cores."""
    x = hs.astype(np.float64)
    rw = np.asarray(inputs["route_w"], np.float64)
    rb = np.asarray(inputs["route_b"], np.float64)
    nw = np.asarray(inputs["noise_w"], np.float64)
    nb = np.asarray(inputs["noise_b"], np.float64)
    lg = x @ rw + rb
    nlg = x @ nw + nb
    ns = np.logaddexp(nlg, 0.0)
    noisy = lg + noise.astype(np.float64) * ns
    order = np.argsort(-noisy, axis=1, kind="stable")
    top2 = order[:, :2]
    rows = np.arange(noisy.shape[0])[:, None]
    v = noisy[rows, top2]  # [T, 2] descending
    e21 = np.exp(v[:, 1] - v[:, 0])
    g1 = 1.0 / (1.0 + e21)
    g2 = e21 / (1.0 + e21)
    gvals = np.stack([g1, g2], axis=1)  # gate at top2 positions
    return top2, gvals


def _make_in_maps(inputs):
    hs, noise, rw_cat, rb_cat, w1, b1r, w2, b2, xT = _marshal(inputs)
    in_maps = []
    for c in range(NCORES):
        sl = slice(c * TL, (c + 1) * TL)
        in_maps.append(
            {
                "xT": np.ascontiguousarray(xT[:, sl]),
                "nz": np.ascontiguousarray(noise[sl]),
                "rw": rw_cat,
                "rb": rb_cat,
                "w1s": w1,
                "b1r": b1r,
                "w2s": w2,
                "b2s": b2,
            }
        )
    return in_maps


def _make_sparse_in_maps(inputs):
    hs, noise, rw_cat, rb_cat, w1, b1r, w2, b2, xT = _marshal(inputs)
    top2, gvals = _host_route(hs, noise, inputs)
    idxs, gvs = [], []
    for e in range(E):
        sel = top2 == e  # [T, 2]
        tok = np.nonzero(sel.any(axis=1))[0]
        which = sel[tok].argmax(axis=1)
        idxs.append(tok)
        gvs.append(gvals[tok, which].astype(np.float32))
    maxn = max(len(i) for i in idxs)
    C = max(1280, -(-maxn // 256) * 256)

    in_maps = []
    for c in range(NCORES):
        sl = slice(c * TL, (c + 1) * TL)
        idx = idxs[c]
        xTg = np.zeros((D, C), np.float32)
        xTg[:, : len(idx)] = xT[:, idx]
        gv = np.zeros((1, C), np.float32)
        gv[0, : len(idx)] = gvs[c]
        in_maps.append(
            {
                "xTr": np.ascontiguousarray(xT[:, sl]),
                "nz": np.ascontiguousarray(noise[sl]),
                "rw": rw_cat,
                "rb": rb_cat,
                "xTg": xTg,
                "gv": gv,
                "w1": np.ascontiguousarray(w1[c]),
                "b1c": np.ascontiguousarray(b1r[:, c, :]),
                "w2": np.ascontiguousarray(w2[c]),
                "b2": b2[c : c + 1],
            }
        )
    return in_maps, idxs, C


def _kernel_dense(inputs):
    nc = _get_nc()
    in_maps = _make_in_maps(inputs)
    res = run_bass_kernel_spmd(nc, in_maps, list(range(NCORES))).results
    out = np.concatenate([res[c]["outp"] for c in range(NCORES)], axis=0)
    gate = np.concatenate([res[c]["gout"] for c in range(NCORES)], axis=0)
    return out.reshape(2, 2048, D), gate


def _kernel_sparse(inputs):
    in_maps, idxs, C = _make_sparse_in_maps(inputs)
    nc = _get_sparse_nc(C)
    res = run_bass_kernel_spmd(nc, in_maps, list(range(NCORES))).results
    out = np.zeros((T, D), np.float32)
    for c in range(NCORES):
        idx = idxs[c]
        out[idx] += res[c]["yout"][: len(idx)]
    gate = np.concatenate([res[c]["gout"] for c in range(NCORES)], axis=0)
    return out.reshape(2, 2048, D), gate


def kernel(**inputs):
    if os.environ.get("MOE_MODE", "sparse") == "dense":
        return _kernel_dense(inputs)
    return _kernel_sparse(inputs)


# revision 12
# speedup vs baseline: 2.0395x; 2.0395x over previous
"""MoE layer (noisy top-2 routing, 8 experts) on 8 TRN2 NeuronCores.

Two device strategies (host picks via MOE_MODE env, default "sparse"):

* sparse (expert parallelism with host-side token dispatch): the host runs the
  router in fp64 purely to decide token->expert placement (the math itself is
  recomputed on device), gathers each expert's ~T/4 tokens, and each core runs
  its expert's FFN on the gathered tokens plus the router for its 512-token
  slice (which produces the returned gate). The host scatter-adds the two
  gate-weighted expert outputs per token. Exploits the top-2 sparsity: 4x
  fewer FLOPs than the dense formulation.

* dense (data parallelism): each core takes T/8 = 512 tokens and runs the
  router + all 8 experts' FFNs on its slice, combining with the on-device
  gate. No host-side math at all; exact replica of the reference dense
  formulation.

Router matmuls run in plain fp32 (top-2 selection needs it: the tightest
2nd/3rd margin is ~2e-5, fp32r noise is ~1e-4). FFN matmuls run in fp32r
(full PE rate, ~1e-4 relative error).

Self-contained: hardcodes shapes from the problem spec.
"""

import os
import sys

import numpy as np

for _p in ("/opt/trn_rl_repo", "/opt/pypackages"):
    if _p not in sys.path:
        sys.path.append(_p)

import concourse.mybir as mybir  # noqa: E402
import concourse.tile as tile  # noqa: E402
from concourse import bacc  # noqa: E402
from concourse.bass_utils import run_bass_kernel_spmd  # noqa: E402

D = 1024  # d_model
H = 4096  # d_ff
E = 8  # experts
NCORES = 8
T = 4096  # total tokens
TL = T // NCORES  # router tokens per core
NT = TL // 128  # router token tiles per core
KD = D // 128  # contraction tiles over d_model
NH = H // 128  # H tiles
F32 = mybir.dt.float32
F32R = mybir.dt.float32r
AX = mybir.AxisListType
ALU = mybir.AluOpType
ACT = mybir.ActivationFunctionType

_cache: dict = {}


def _emit_router(nc, apool, pspool, xT_sb, rw_sb, rb_sb, nz_sb, gate_sb, gout_v):
    """Noisy top-2 router for NT tiles of 128 tokens; writes gate to gout and
    gate_sb. Plain fp32 matmuls."""
    for t in range(NT):
        ps_r = pspool.tile(
            [128, 2 * E], F32, name="ps", tag="ps", padded_shape=[128, 512]
        )
        for k in range(KD):
            nc.tensor.matmul(
                ps_r[:],
                xT_sb[:, k, t * 128 : (t + 1) * 128],
                rw_sb[:, k, :],
                start=(k == 0),
                stop=(k == KD - 1),
            )
        lg = apool.tile([128, 2 * E], F32, name="lg", tag="lg")
        nc.vector.tensor_add(lg[:], ps_r[:], rb_sb[:])

        # softplus(z) = relu(z) + ln(1 + exp(-|z|)) via exp/ln tables
        az = apool.tile([128, E], F32, name="az", tag="az")
        nc.scalar.activation(az[:], lg[:, E:], ACT.Abs)
        en = apool.tile([128, E], F32, name="en", tag="en")
        nc.scalar.activation(en[:], az[:], ACT.Exp, scale=-1.0)
        nc.vector.tensor_scalar(en[:], en[:], 1.0, None, ALU.add)
        lnv = apool.tile([128, E], F32, name="lnv", tag="lnv")
        nc.scalar.activation(lnv[:], en[:], ACT.Ln)
        rz = apool.tile([128, E], F32, name="rz", tag="rz")
        nc.scalar.activation(rz[:], lg[:, E:], ACT.Relu)
        ns = apool.tile([128, E], F32, name="ns", tag="ns")
        nc.vector.tensor_add(ns[:], lnv[:], rz[:])
        noisy = apool.tile([128, E], F32, name="noisy", tag="noisy")
        nc.vector.tensor_mul(noisy[:], nz_sb[:, t, :], ns[:])
        nc.vector.tensor_add(noisy[:], noisy[:], lg[:, :E])

        m1 = apool.tile([128, 1], F32, name="m1", tag="m1")
        nc.vector.reduce_max(m1[:], noisy[:], axis=AX.X)
        eq = apool.tile([128, E], F32, name="eq", tag="eq")
        nc.vector.tensor_scalar(eq[:], noisy[:], m1[:], None, ALU.is_equal)
        v2 = apool.tile([128, E], F32, name="v2", tag="v2")
        nc.vector.scalar_tensor_tensor(
            v2[:], eq[:], -1e30, noisy[:], op0=ALU.mult, op1=ALU.add
        )
        m2 = apool.tile([128, 1], F32, name="m2", tag="m2")
        nc.vector.reduce_max(m2[:], v2[:], axis=AX.X)

        negm1 = apool.tile([128, 1], F32, name="negm1", tag="negm1")
        nc.vector.tensor_scalar(negm1[:], m1[:], -1.0, None, ALU.mult)
        expv = apool.tile([128, E], F32, name="expv", tag="expv")
        nc.scalar.activation(expv[:], noisy[:], ACT.Exp, bias=negm1[:])
        # r = sigmoid(m1 - m2) = 1 / (1 + exp(m2 - m1))
        d21 = apool.tile([128, 1], F32, name="d21", tag="d21")
        nc.vector.tensor_sub(d21[:], m2[:], m1[:])
        er = apool.tile([128, 1], F32, name="er", tag="er")
        nc.scalar.activation(er[:], d21[:], ACT.Exp)
        nc.vector.tensor_scalar(er[:], er[:], 1.0, None, ALU.add)
        r = apool.tile([128, 1], F32, name="r", tag="r")
        nc.vector.reciprocal(r[:], er[:])
        ge = apool.tile([128, E], F32, name="ge", tag="ge")
        nc.vector.tensor_scalar(ge[:], noisy[:], m2[:], None, ALU.is_ge)
        nc.vector.tensor_mul(expv[:], expv[:], ge[:])
        nc.vector.tensor_scalar(gate_sb[:, t, :], expv[:], r[:], None, ALU.mult)

        nc.sync.dma_start(gout_v[t], gate_sb[:, t, :])


def _chunks(C):
    out, off = [], 0
    while off < C:
        w = min(512, C - off)
        out.append((off, w))
        off += w
    return out


def _build_sparse_nc(C, reps=1):
    """SPMD program: router on a 512-token slice + one expert's FFN on C
    pre-gathered tokens. reps>1 repeats the compute body (bench only)."""
    nc = bacc.Bacc(None, target_bir_lowering=False)

    xTr = nc.dram_tensor("xTr", [D, TL], F32, kind="ExternalInput")
    nz = nc.dram_tensor("nz", [TL, E], F32, kind="ExternalInput")
    rw = nc.dram_tensor("rw", [D, 2 * E], F32, kind="ExternalInput")
    rb = nc.dram_tensor("rb", [128, 2 * E], F32, kind="ExternalInput")
    xTg = nc.dram_tensor("xTg", [D, C], F32R, kind="ExternalInput")
    gv = nc.dram_tensor("gv", [1, C], F32R, kind="ExternalInput")
    w1 = nc.dram_tensor("w1", [D, H], F32R, kind="ExternalInput")
    b1c = nc.dram_tensor("b1c", [128, NH], F32, kind="ExternalInput")
    w2 = nc.dram_tensor("w2", [H, D], F32R, kind="ExternalInput")
    b2 = nc.dram_tensor("b2", [1, D], F32R, kind="ExternalInput")
    yout = nc.dram_tensor("yout", [C, D], F32, kind="ExternalOutput")
    gout = nc.dram_tensor("gout", [TL, E], F32, kind="ExternalOutput")

    yout_v = yout.rearrange("(t p) d -> t p d", p=128)
    gout_v = gout.rearrange("(t p) e -> t p e", p=128)
    CT = C // 128  # gathered token tiles

    with tile.TileContext(nc) as tc:
        with (
            tc.tile_pool(name="const", bufs=1) as cpool,
            tc.tile_pool(name="acts", bufs=1) as apool,
            tc.tile_pool(name="hbuf", bufs=36) as hpool,
            tc.tile_pool(name="w1buf", bufs=4) as w1pool,
            tc.tile_pool(name="w2buf", bufs=4) as w2pool,
            tc.tile_pool(name="ybuf", bufs=4) as ypool,
            tc.tile_pool(name="ps", bufs=8, space="PSUM") as pspool,
        ):
            xT_sb = apool.tile([128, KD, TL], F32)
            nc.sync.dma_start(xT_sb[:], xTr.rearrange("(k p) t -> p k t", p=128))
            rw_sb = cpool.tile([128, KD, 2 * E], F32)
            nc.sync.dma_start(rw_sb[:], rw.rearrange("(k p) c -> p k c", p=128))
            rb_sb = cpool.tile([128, 2 * E], F32)
            nc.sync.dma_start(rb_sb[:], rb[:])
            nz_sb = apool.tile([128, NT, E], F32)
            nc.sync.dma_start(nz_sb[:], nz.rearrange("(t p) e -> p t e", p=128))
            gate_sb = apool.tile([128, NT, E], F32)

            xTg_sb = apool.tile([128, KD, C], F32R)
            nc.sync.dma_start(xTg_sb[:], xTg.rearrange("(k p) t -> p k t", p=128))
            gvc_sb = apool.tile([128, CT], F32)
            nc.sync.dma_start(
                gvc_sb[:], gv.bitcast(F32).rearrange("o (t p) -> (o p) t", p=128)
            )
            gvr_sb = cpool.tile([1, C], F32R)
            nc.sync.dma_start(gvr_sb[:], gv[:])
            b1_sb = cpool.tile([128, NH], F32)
            nc.sync.dma_start(b1_sb[:], b1c[:])
            b2_sb = cpool.tile([1, D], F32R)
            nc.sync.dma_start(b2_sb[:], b2[:])

            for _rep in range(reps):
                _emit_router(
                    nc, apool, pspool, xT_sb, rw_sb, rb_sb, nz_sb, gate_sb, gout_v
                )
                _emit_sparse_ffn(
                    nc, C, pspool, hpool, w1pool, w2pool, ypool,
                    xTg_sb, gvc_sb, gvr_sb, b1_sb, b2_sb, w1, w2, yout_v,
                )

    nc.compile()
    return nc


def _emit_sparse_ffn(
    nc, C, pspool, hpool, w1pool, w2pool, ypool,
    xTg_sb, gvc_sb, gvr_sb, b1_sb, b2_sb, w1, w2, yout_v,
):
    if True:
            for coff, cw in _chunks(C):
                ct0 = coff // 128
                # ---- h^T = relu(w1.T @ xTg + b1) for this token chunk ----
                h_tiles = []
                for g in range(NH // 4):
                    psh = [
                        pspool.tile([128, 512], F32, name="ps", tag="ps")
                        for _ in range(4)
                    ]
                    for k in range(KD):
                        w1t = w1pool.tile([128, 512], F32R, name="w1t", tag="w1t")
                        nc.sync.dma_start(
                            w1t[:],
                            w1[k * 128 : (k + 1) * 128, g * 512 : (g + 1) * 512],
                        )
                        for i4 in range(4):
                            nc.tensor.matmul(
                                psh[i4][:, :cw],
                                w1t[:, i4 * 128 : (i4 + 1) * 128],
                                xTg_sb[:, k, coff : coff + cw],
                                start=(k == 0),
                                stop=(k == KD - 1),
                            )
                    for i4 in range(4):
                        i = g * 4 + i4
                        ht = hpool.tile([128, 512], F32R, name="ht", tag="ht")
                        nc.scalar.activation(
                            ht[:, :cw], psh[i4][:, :cw], ACT.Relu,
                            bias=b1_sb[:, i : i + 1],
                        )
                        h_tiles.append(ht)

                # ---- y = gv * (h @ w2 + b2), token-major ----
                ntt = cw // 128
                for d in range(2):
                    psy = [
                        pspool.tile([128, 512], F32, name="ps", tag="ps")
                        for _ in range(ntt)
                    ]
                    for i in range(NH):
                        w2t = w2pool.tile([128, 512], F32R, name="w2t", tag="w2t")
                        nc.sync.dma_start(
                            w2t[:],
                            w2[i * 128 : (i + 1) * 128, d * 512 : (d + 1) * 512],
                        )
                        for tt in range(ntt):
                            nc.tensor.matmul(
                                psy[tt][:],
                                h_tiles[i][:, tt * 128 : (tt + 1) * 128],
                                w2t[:],
                                start=(i == 0),
                                stop=False,
                            )
                    for tt in range(ntt):
                        # gated b2 term: psy += gv[:,tile].T @ b2 (rank-1, fp32)
                        nc.tensor.matmul(
                            psy[tt][:],
                            gvr_sb[:, coff + tt * 128 : coff + (tt + 1) * 128],
                            b2_sb[:, d * 512 : (d + 1) * 512],
                            start=False,
                            stop=True,
                        )
                        y_sb = ypool.tile([128, 512], F32, name="y_sb", tag="y_sb")
                        nc.vector.tensor_scalar(
                            y_sb[:], psy[tt][:], gvc_sb[:, ct0 + tt : ct0 + tt + 1],
                            None, ALU.mult,
                        )
                        nc.sync.dma_start(
                            yout_v[ct0 + tt][:, d * 512 : (d + 1) * 512], y_sb[:]
                        )


def _build_dense_nc():
    """SPMD program: router + dense 8-expert FFN for TL tokens."""
    nc = bacc.Bacc(None, target_bir_lowering=False)

    xT = nc.dram_tensor("xT", [D, TL], F32R, kind="ExternalInput")
    nz = nc.dram_tensor("nz", [TL, E], F32, kind="ExternalInput")
    rw = nc.dram_tensor("rw", [D, 2 * E], F32, kind="ExternalInput")
    rb = nc.dram_tensor("rb", [128, 2 * E], F32, kind="ExternalInput")
    w1s = nc.dram_tensor("w1s", [E, D, H], F32R, kind="ExternalInput")
    b1r = nc.dram_tensor("b1r", [128, E, NH], F32, kind="ExternalInput")
    w2s = nc.dram_tensor("w2s", [E, H, D], F32R, kind="ExternalInput")
    b2s = nc.dram_tensor("b2s", [E, D], F32, kind="ExternalInput")
    outp = nc.dram_tensor("outp", [TL, D], F32, kind="ExternalOutput")
    gout = nc.dram_tensor("gout", [TL, E], F32, kind="ExternalOutput")

    out_v = outp.rearrange("(t p) d -> t p d", p=128)
    gout_v = gout.rearrange("(t p) e -> t p e", p=128)

    with tile.TileContext(nc) as tc:
        with (
            tc.tile_pool(name="const", bufs=1) as cpool,
            tc.tile_pool(name="acts", bufs=1) as apool,
            tc.tile_pool(name="hbuf", bufs=36) as hpool,
            tc.tile_pool(name="w1buf", bufs=3) as w1pool,
            tc.tile_pool(name="w2buf", bufs=3) as w2pool,
            tc.tile_pool(name="accbuf", bufs=8) as accpool,
            tc.tile_pool(name="ps", bufs=8, space="PSUM") as pspool,
        ):
            from concourse.masks import make_identity

            xT_sb = apool.tile([128, KD, TL], F32)
            nc.sync.dma_start(
                xT_sb[:], xT.bitcast(F32).rearrange("(k p) t -> p k t", p=128)
            )
            xTr_sb = apool.tile([128, KD, TL], F32R)
            nc.sync.dma_start(xTr_sb[:], xT.rearrange("(k p) t -> p k t", p=128))
            rw_sb = cpool.tile([128, KD, 2 * E], F32)
            nc.sync.dma_start(rw_sb[:], rw.rearrange("(k p) c -> p k c", p=128))
            rb_sb = cpool.tile([128, 2 * E], F32)
            nc.sync.dma_start(rb_sb[:], rb[:])
            nz_sb = apool.tile([128, NT, E], F32)
            nc.sync.dma_start(nz_sb[:], nz.rearrange("(t p) e -> p t e", p=128))
            b1_sb = cpool.tile([128, E, NH], F32)
            nc.sync.dma_start(b1_sb[:], b1r[:])
            b2_sb = cpool.tile([E, D], F32)
            nc.sync.dma_start(b2_sb[:], b2s[:])
            ident = cpool.tile([128, 128], F32)
            make_identity(nc, ident[:])

            gate_sb = apool.tile([128, NT, E], F32)
            gT_sb = apool.tile([E, TL], F32)

            _emit_router(nc, apool, pspool, xT_sb, rw_sb, rb_sb, nz_sb, gate_sb, gout_v)

            # gate transposed (for the gate @ b2 acc init)
            for t in range(NT):
                ps_gt = pspool.tile(
                    [E, 128], F32, name="ps", tag="ps", padded_shape=[128, 512]
                )
                nc.tensor.transpose(ps_gt[:], gate_sb[:, t, :], ident[:])
                nc.vector.tensor_copy(gT_sb[:, t * 128 : (t + 1) * 128], ps_gt[:])

            # ---- acc init with the gate-weighted b2 term: acc = gate @ b2s ----
            acc = {}
            for t in range(NT):
                for d in range(2):
                    ps = pspool.tile([128, 512], F32, name="ps", tag="ps")
                    nc.tensor.matmul(
                        ps[:],
                        gT_sb[:, t * 128 : (t + 1) * 128],
                        b2_sb[:, d * 512 : (d + 1) * 512],
                        start=True,
                        stop=True,
                    )
                    a = accpool.tile([128, 512], F32, name="a", tag="a")
                    nc.vector.tensor_copy(a[:], ps[:])
                    acc[(t, d)] = a

            # ---- experts ----
            for e in range(E):
                h_tiles = []
                for g in range(NH // 4):  # groups of 4 H-tiles (512 cols of w1)
                    psh = [
                        pspool.tile([128, 512], F32, name="ps", tag="ps")
                        for _ in range(4)
                    ]
                    for k in range(KD):
                        w1t = w1pool.tile([128, 512], F32R, name="w1t", tag="w1t")
                        nc.sync.dma_start(
                            w1t[:],
                            w1s[e, k * 128 : (k + 1) * 128, g * 512 : (g + 1) * 512],
                        )
                        for i4 in range(4):
                            nc.tensor.matmul(
                                psh[i4][:],
                                w1t[:, i4 * 128 : (i4 + 1) * 128],
                                xTr_sb[:, k, :],
                                start=(k == 0),
                                stop=(k == KD - 1),
                            )
                    for i4 in range(4):
                        i = g * 4 + i4
                        ht = hpool.tile([128, TL], F32R, name="ht", tag="ht")
                        nc.scalar.activation(
                            ht[:], psh[i4][:], ACT.Relu, bias=b1_sb[:, e, i : i + 1]
                        )
                        h_tiles.append(ht)

                for d in range(2):
                    psy = [
                        pspool.tile([128, 512], F32, name="ps", tag="ps")
                        for _ in range(NT)
                    ]
                    for i in range(NH):
                        w2t = w2pool.tile([128, 512], F32R, name="w2t", tag="w2t")
                        nc.sync.dma_start(
                            w2t[:],
                            w2s[e, i * 128 : (i + 1) * 128, d * 512 : (d + 1) * 512],
                        )
                        for t in range(NT):
                            nc.tensor.matmul(
                                psy[t][:],
                                h_tiles[i][:, t * 128 : (t + 1) * 128],
                                w2t[:],
                                start=(i == 0),
                                stop=(i == NH - 1),
                            )
                    for t in range(NT):
                        nc.vector.scalar_tensor_tensor(
                            acc[(t, d)][:],
                            psy[t][:],
                            gate_sb[:, t, e : e + 1],
                            acc[(t, d)][:],
                            op0=ALU.mult,
                            op1=ALU.add,
                        )

            for t in range(NT):
                for d in range(2):
                    nc.sync.dma_start(
                        out_v[t][:, d * 512 : (d + 1) * 512], acc[(t, d)][:]
                    )

    nc.compile()
    return nc


def _get_nc():
    if "dense" not in _cache:
        _cache["dense"] = _build_dense_nc()
    return _cache["dense"]


def _get_sparse_nc(C):
    key = ("sparse", C)
    if key not in _cache:
        _cache[key] = _build_sparse_nc(C)
    return _cache[key]


def _marshal(inputs):
    f = lambda k: np.ascontiguousarray(np.asarray(inputs[k], dtype=np.float32))
    hs = f("hidden_states").reshape(T, D)
    noise = f("noise")
    rw_cat = np.concatenate([f("route_w"), f("noise_w")], axis=1)  # [D, 16]
    rb_cat = np.broadcast_to(
        np.concatenate([f("route_b"), f("noise_b")])[None, :], (128, 2 * E)
    ).copy()
    w1 = f("w1")
    b1r = f("b1").reshape(E, NH, 128).transpose(2, 0, 1).copy()  # [128, E, NH]
    w2 = f("w2")
    b2 = f("b2")
    xT = np.ascontiguousarray(hs.T)  # [D, T]
    return hs, noise, rw_cat, rb_cat, w1, b1r, w2, b2, xT


def _host_route(hs, noise, inputs):
    """fp64 replica of the router, used only to place tokens on cores."""
    x = hs.astype(np.float64)
    rw = np.asarray(inputs["route_w"], np.float64)
    rb = np.asarray(inputs["route_b"], np.float64)
    nw = np.asarray(inputs["noise_w"], np.float64)
    nb = np.asarray(inputs["noise_b"], np.float64)
    lg = x @ rw + rb
    nlg = x @ nw + nb
    ns = np.logaddexp(nlg, 0.0)
    noisy = lg + noise.astype(np.float64) * ns
    order = np.argsort(-noisy, axis=1, kind="stable")
    top2 = order[:, :2]
    rows = np.arange(noisy.shape[0])[:, None]
    v = noisy[rows, top2]  # [T, 2] descending
    e21 = np.exp(v[:, 1] - v[:, 0])
    g1 = 1.0 / (1.0 + e21)
    g2 = e21 / (1.0 + e21)
    gvals = np.stack([g1, g2], axis=1)  # gate at top2 positions
    return top2, gvals


def _make_in_maps(inputs):
    hs, noise, rw_cat, rb_cat, w1, b1r, w2, b2, xT = _marshal(inputs)
    in_maps = []
    for c in range(NCORES):
        sl = slice(c * TL, (c + 1) * TL)
        in_maps.append(
            {
                "xT": np.ascontiguousarray(xT[:, sl]),
                "nz": np.ascontiguousarray(noise[sl]),
                "rw": rw_cat,
                "rb": rb_cat,
                "w1s": w1,
                "b1r": b1r,
                "w2s": w2,
                "b2s": b2,
            }
        )
    return in_maps


def _make_sparse_in_maps(inputs):
    hs, noise, rw_cat, rb_cat, w1, b1r, w2, b2, xT = _marshal(inputs)
    top2, gvals = _host_route(hs, noise, inputs)
    idxs, gvs = [], []
    for e in range(E):
        sel = top2 == e  # [T, 2]
        tok = np.nonzero(sel.any(axis=1))[0]
        which = sel[tok].argmax(axis=1)
        idxs.append(tok)
        gvs.append(gvals[tok, which].astype(np.float32))
    maxn = max(len(i) for i in idxs)
    C = max(1280, -(-maxn // 256) * 256)

    in_maps = []
    for c in range(NCORES):
        sl = slice(c * TL, (c + 1) * TL)
        idx = idxs[c]
        xTg = np.zeros((D, C), np.float32)
        xTg[:, : len(idx)] = xT[:, idx]
        gv = np.zeros((1, C), np.float32)
        gv[0, : len(idx)] = gvs[c]
        in_maps.append(
            {
                "xTr": np.ascontiguousarray(xT[:, sl]),
                "nz": np.ascontiguousarray(noise[sl]),
                "rw": rw_cat,
                "rb": rb_cat,
                "xTg": xTg,
                "gv": gv,
                "w1": np.ascontiguousarray(w1[c]),
                "b1c": np.ascontiguousarray(b1r[:, c, :]),
                "w2": np.ascontiguousarray(w2[c]),
                "b2": b2[c : c + 1],
            }
        )
    return in_maps, idxs, C


def _kernel_dense(inputs):
    nc = _get_nc()
    in_maps = _make_in_maps(inputs)
    res = run_bass_kernel_spmd(nc, in_maps, list(range(NCORES))).results
    out = np.concatenate([res[c]["outp"] for c in range(NCORES)], axis=0)
    gate = np.concatenate([res[c]["gout"] for c in range(NCORES)], axis=0)
    return out.reshape(2, 2048, D), gate


def _kernel_sparse(inputs):
    in_maps, idxs, C = _make_sparse_in_maps(inputs)
    nc = _get_sparse_nc(C)
    res = run_bass_kernel_spmd(nc, in_maps, list(range(NCORES))).results
    out = np.zeros((T, D), np.float32)
    for c in range(NCORES):
        idx = idxs[c]
        out[idx] += res[c]["yout"][: len(idx)]
    gate = np.concatenate([res[c]["gout"] for c in range(NCORES)], axis=0)
    return out.reshape(2, 2048, D), gate


def kernel(**inputs):
    if os.environ.get("MOE_MODE", "sparse") == "dense":
        return _kernel_dense(inputs)
    return _kernel_sparse(inputs)
